# revision 14
# baseline (speedup 1.0000x reference)
"""DeformableConv2dGuided Trainium2 kernel (8 NeuronCores, data-parallel b x row-half).

Per core (core = (b, half)):
  - 96 output rows; flat pixel space uses pitch WP=194 (leading col maps x=0 to col 0,
    cols 192/193 are zero-gap garbage), PX = 96*194, padded to PXP = 147*128 = 18816.
  - conv: 18 PSUM-accumulating matmuls (ky,kx,chunk) bf16, M=12 (8 off + 4 mod).
  - fields/idx in pixel-partition layout [128,147] (px i at (i%128, i//128)).
  - gather: 512B tokens [g00|g01|g10|g11] x 64ch bf16 from host-built padded q array,
    via gpsimd.dma_gather (transpose). G[128, 2, n]: partitions<64 ch of x-tap0,
    >=64 x-tap1; free slot = y-tap.
  - weight tiles W0=[w00;w01], W1=[w10;w11] via partition_broadcast(channels=64)
    + partition-shifted copy of the second field into the bottom half.
  - contraction: lhsT = [w_reg.T; w_reg.T]*0.5, PSUM accumulates over 4 refs x 2
    slots -> folds modulated mean + 1x1 conv into one pass.
"""
import os
import numpy as np
import ml_dtypes

import concourse.bacc as bacc
import concourse.mybir as mybir
from concourse.tile import TileContext
from concourse.bass_utils import run_bass_kernel_spmd

F32 = mybir.dt.float32
BF16 = mybir.dt.bfloat16
I16 = mybir.dt.int16
I32 = mybir.dt.int32
ALU = mybir.AluOpType
ACTF = mybir.ActivationFunctionType

B, N, C, H, W = 4, 4, 64, 192, 192
RH = 96                 # rows per core
WP = 194                # pitched row width
PX = RH * WP            # 18624
PXP = 147 * 128         # 18816 padded
NCHUNK = 7
NI = PXP // NCHUNK      # 2688 idxs per gather chunk (21*128)
PAD = 6                 # token array pad around image
TW = W + 2 * PAD        # 204 token cols
TROWS = RH + 2 * PAD    # 108 token rows per core
TN = TROWS * TW         # tokens per (core, ref)
XCROWS = RH + 2
XCF = XCROWS * WP       # 19012
XCFP = 19456
CBLK = [512] * 36 + [384]


def _emit_out_dmas(nc, out_d, src_tile, s, e, nch):
    """DMA src_tile[:, 0:e-s] (flat pitched px [s,e)) to out_d[nch, RH*W] row-major,
    skipping the 2 garbage cols per row."""
    p = s
    e = min(e, PX)
    while p < e:
        y = p // WP
        x0 = p - y * WP
        x1 = min(WP, x0 + (e - p))
        cx0, cx1 = x0, min(x1, W)
        if cx1 > cx0:
            nc.sync.dma_start(
                out_d[:, y * W + cx0: y * W + cx1],
                src_tile[:, p - s + (cx0 - x0): p - s + (cx1 - x0)],
            )
        p += x1 - x0


def _build_program(b_off, b_mod):
    nc = bacc.Bacc("TRN2", target_bir_lowering=False, debug=False, num_devices=8)

    xc_d = nc.dram_tensor("xc", [2, 128, XCF], BF16, kind="ExternalInput")
    q_d = nc.dram_tensor("q", [N, TN, 256], BF16, kind="ExternalInput")
    wconv_d = nc.dram_tensor("wconv", [128, 216], BF16, kind="ExternalInput")
    wreg_d = nc.dram_tensor("wreg", [128, 64], BF16, kind="ExternalInput")
    gy_d = nc.dram_tensor("gy", [128, 147], F32, kind="ExternalInput")
    gx_d = nc.dram_tensor("gx", [128, 147], F32, kind="ExternalInput")
    out_d = nc.dram_tensor("out", [64, RH * W], F32, kind="ExternalOutput")
    off_d = nc.dram_tensor("off", [8, RH * W], F32, kind="ExternalOutput")

    with TileContext(nc) as tc:
        with (
            tc.tile_pool(name="const", bufs=1) as const_pool,
            tc.tile_pool(name="convs", bufs=2) as convs_pool,
            tc.tile_pool(name="psum", bufs=2, space="PSUM") as psum_pool,
            tc.tile_pool(name="fld", bufs=1) as fld_pool,
            tc.tile_pool(name="tmp", bufs=2) as tmp_pool,
            tc.tile_pool(name="dram", bufs=1, space="DRAM") as dram_pool,
        ):
            wconv_t = const_pool.tile([128, 216], BF16)
            nc.sync.dma_start(wconv_t[:, :], wconv_d[:, :])
            wreg_t = const_pool.tile([128, 64], BF16)
            nc.sync.dma_start(wreg_t[:, :], wreg_d[:, :])
            gy_t = const_pool.tile([128, 147], F32)
            nc.sync.dma_start(gy_t[:, :], gy_d[:, :])
            gx_t = const_pool.tile([128, 147], F32)
            nc.sync.dma_start(gx_t[:, :], gx_d[:, :])

            scr_c = dram_pool.tile([12, PXP], F32)
            scr_i = dram_pool.tile([4, PXP], I16)
            scr_w = dram_pool.tile([4, 4, PXP], BF16)
            scr_o = dram_pool.tile([8, PXP], F32)

            # ---------------- Stage A: conv ----------------
            with tc.tile_pool(name="xc", bufs=1) as xc_pool:
                xct = []
                for ch in range(2):
                    t = xc_pool.tile([128, XCFP], BF16, tag=f"xc{ch}")
                    nc.vector.memset(t[:, XCF:], 0.0)
                    nc.sync.dma_start(t[:, :XCF], xc_d[ch, :, :])
                    xct.append(t)

                pos = 0
                for blk_len in CBLK:
                    ps = psum_pool.tile([12, 512], F32, tag="convps")
                    nmm = 0
                    for ch in range(2):
                        for ky in range(3):
                            for kx in range(3):
                                dlt = ky * WP + kx
                                nc.tensor.matmul(
                                    ps[:, :blk_len],
                                    wconv_t[:, 108 * ch + 12 * (3 * ky + kx):
                                            108 * ch + 12 * (3 * ky + kx) + 12],
                                    xct[ch][:, pos + dlt:pos + dlt + blk_len],
                                    start=(nmm == 0),
                                    stop=(nmm == 17),
                                )
                                nmm += 1
                    cvt = convs_pool.tile([12, 512], F32, tag="cvt")
                    nc.vector.tensor_copy(cvt[:, :blk_len], ps[:, :blk_len])
                    nc.sync.dma_start(scr_c[:, pos:pos + blk_len], cvt[:, :blk_len])
                    pos += blk_len

            # ---------------- Stage B: conv rows -> pixel-partition ----------------
            cpp = fld_pool.tile([128, 12, 147], F32)
            for r in range(12):
                nc.sync.dma_start(cpp[:, r, :], scr_c[r, :].rearrange("(f p) -> p f", p=128))

            # ---------------- Stage C: field math per ref ----------------
            for n in range(N):
                dyr = cpp[:, 2 * n, :]
                dxr = cpp[:, 2 * n + 1, :]
                mr = cpp[:, 8 + n, :]

                offy = tmp_pool.tile([128, 147], F32, tag="offy")
                offx = tmp_pool.tile([128, 147], F32, tag="offx")
                nc.vector.tensor_scalar(offy[:, :], dyr, float(b_off[2 * n]), -1.0, ALU.add, ALU.mult)
                nc.vector.tensor_scalar(offx[:, :], dxr, float(b_off[2 * n + 1]), -1.0, ALU.add, ALU.mult)
                nc.sync.dma_start(scr_o[2 * n, :].rearrange("(f p) -> p f", p=128), offy[:, :])
                nc.sync.dma_start(scr_o[2 * n + 1, :].rearrange("(f p) -> p f", p=128), offx[:, :])

                py = tmp_pool.tile([128, 147], F32, tag="py")
                pxt = tmp_pool.tile([128, 147], F32, tag="pxt")
                nc.vector.tensor_tensor(py[:, :], gy_t[:, :], offy[:, :], ALU.add)
                nc.vector.tensor_tensor(pxt[:, :], gx_t[:, :], offx[:, :], ALU.add)

                def floor_of(v, tag):
                    t_s = tmp_pool.tile([128, 147], F32, tag=f"fs{tag}")
                    t_i = tmp_pool.tile([128, 147], I32, tag=f"fi{tag}")
                    t_f = tmp_pool.tile([128, 147], F32, tag=f"ff{tag}")
                    nc.vector.tensor_scalar(t_s[:, :], v, 16.0, None, ALU.add)
                    nc.vector.tensor_copy(t_i[:, :], t_s[:, :])
                    nc.vector.tensor_copy(t_f[:, :], t_i[:, :])
                    y0 = tmp_pool.tile([128, 147], F32, tag=f"fl{tag}")
                    nc.vector.tensor_scalar(y0[:, :], t_f[:, :], -16.0, None, ALU.add)
                    fr = tmp_pool.tile([128, 147], F32, tag=f"fr{tag}")
                    nc.vector.tensor_tensor(fr[:, :], v, y0[:, :], ALU.subtract)
                    msk = tmp_pool.tile([128, 147], F32, tag=f"fm{tag}")
                    nc.vector.tensor_scalar(msk[:, :], fr[:, :], 0.0, None, ALU.is_lt)
                    nc.vector.tensor_tensor(y0[:, :], y0[:, :], msk[:, :], ALU.subtract)
                    nc.vector.tensor_tensor(fr[:, :], fr[:, :], msk[:, :], ALU.add)
                    return y0, fr

                y0, fy = floor_of(py[:, :], "y")
                x0, fx = floor_of(pxt[:, :], "x")

                y0c = tmp_pool.tile([128, 147], F32, tag="y0c")
                x0c = tmp_pool.tile([128, 147], F32, tag="x0c")
                nc.vector.tensor_scalar(y0c[:, :], y0[:, :], -float(PAD), float(RH + PAD - 1), ALU.max, ALU.min)
                nc.vector.tensor_scalar(x0c[:, :], x0[:, :], -float(PAD), float(W + PAD - 1), ALU.max, ALU.min)
                idxf = tmp_pool.tile([128, 147], F32, tag="idxf")
                nc.vector.scalar_tensor_tensor(idxf[:, :], y0c[:, :], float(TW), x0c[:, :], ALU.mult, ALU.add)
                nc.vector.tensor_scalar(idxf[:, :], idxf[:, :], float(PAD * TW + PAD), None, ALU.add)
                idx16 = tmp_pool.tile([128, 147], I16, tag="idx16")
                nc.vector.tensor_copy(idx16[:, :], idxf[:, :])
                nc.sync.dma_start(scr_i[n, :].rearrange("(f p) -> p f", p=128), idx16[:, :])

                sig = tmp_pool.tile([128, 147], F32, tag="sig")
                bmod_t = tmp_pool.tile([128, 1], F32, tag="bmod")
                nc.vector.memset(bmod_t[:, :], float(b_mod[n]))
                nc.scalar.activation(sig[:, :], mr, ACTF.Sigmoid, bias=bmod_t[:, :])

                afld = tmp_pool.tile([128, 147], F32, tag="afld")
                bfld = tmp_pool.tile([128, 147], F32, tag="bfld")
                ur = tmp_pool.tile([128, 147], F32, tag="ur")
                nc.vector.tensor_scalar(afld[:, :], fy[:, :], -1.0, 1.0, ALU.mult, ALU.add)
                nc.vector.tensor_tensor(afld[:, :], afld[:, :], sig[:, :], ALU.mult)
                nc.vector.tensor_tensor(bfld[:, :], fy[:, :], sig[:, :], ALU.mult)
                nc.vector.tensor_scalar(ur[:, :], fx[:, :], -1.0, 1.0, ALU.mult, ALU.add)

                for k, (yf, xf) in enumerate(((afld, ur), (afld, fx), (bfld, ur), (bfld, fx))):
                    wk = tmp_pool.tile([128, 147], BF16, tag=f"wk{k}")
                    nc.vector.tensor_tensor(wk[:, :], yf[:, :], xf[:, :], ALU.mult)
                    nc.sync.dma_start(scr_w[n, k, :].rearrange("(f p) -> p f", p=128), wk[:, :])

            # wrapped idx per ref: idx i at (i%16 + 16g, i//16)
            idxw_refs = []
            for n in range(N):
                t = fld_pool.tile([128, PXP // 16], I16, tag=f"idxw{n}")
                for g in range(8):
                    nc.sync.dma_start(
                        t[16 * g:16 * (g + 1), :],
                        scr_i[n, :].rearrange("(f p) -> p f", p=16),
                    )
                idxw_refs.append(t)

            # offsets to DRAM output (strip gap cols)
            for r in range(8):
                nc.sync.dma_start(
                    off_d[r, :].rearrange("(y w) -> y w", w=W),
                    scr_o[r, :PX].rearrange("(y w) -> y w", w=WP)[:, 0:W],
                )

            # ---------------- Stage E: gather + products + contraction ----------------
            with (
                tc.tile_pool(name="gW", bufs=2) as gw_pool,
                tc.tile_pool(name="G", bufs=2) as g_pool,
                tc.tile_pool(name="P", bufs=1) as p_pool,
                tc.tile_pool(name="outc", bufs=1) as outc_pool,
            ):
                for chunk in range(NCHUNK):
                    base = chunk * NI
                    Ps = []
                    for ref in range(N):
                        w_rows = [gw_pool.tile([1, NI], BF16, tag=f"wrow{k}", name=f"wrow{k}") for k in range(4)]
                        for k in range(4):
                            nc.sync.dma_start(w_rows[k][:, :], scr_w[ref, k, base:base + NI])
                        W0 = gw_pool.tile([128, NI], BF16, tag="W0")
                        W1 = gw_pool.tile([128, NI], BF16, tag="W1")
                        tmpb = gw_pool.tile([128, NI], BF16, tag="tmpb")
                        if os.environ.get("KSTAGE_NOBCAST"):
                            nc.vector.memset(W0[:, :], 0.25)
                            nc.vector.memset(W1[:, :], 0.25)
                        else:
                            nc.gpsimd.partition_broadcast(W0[:, :], w_rows[0][:, :], channels=64)
                            nc.gpsimd.partition_broadcast(tmpb[:, :], w_rows[1][:, :], channels=64)
                            nc.scalar.copy(W0[64:128, :], tmpb[0:64, :])
                            nc.gpsimd.partition_broadcast(W1[:, :], w_rows[2][:, :], channels=64)
                            nc.gpsimd.partition_broadcast(tmpb[:, :], w_rows[3][:, :], channels=64)
                            nc.scalar.copy(W1[64:128, :], tmpb[0:64, :])

                        G = g_pool.tile([128, 2 * NI], BF16, tag="G")
                        if os.environ.get("KSTAGE_NOGATHER"):
                            nc.vector.memset(G[:, :], 0.0)
                        else:
                            nc.gpsimd.dma_gather(
                            G[:, :].rearrange("p (s i) -> p s i", s=2),
                            q_d[ref, :, :],
                            idxw_refs[ref][:, chunk * (NI // 16):(chunk + 1) * (NI // 16)],
                                num_idxs=NI, num_idxs_reg=NI, elem_size=256, transpose=True,
                                single_packet=False,
                            )
                        P0 = p_pool.tile([128, NI], BF16, tag=f"P0_{ref}")
                        P1 = p_pool.tile([128, NI], BF16, tag=f"P1_{ref}")
                        nc.vector.tensor_tensor(P0[:, :], W0[:, :], G[:, 0:NI], ALU.mult)
                        nc.vector.tensor_tensor(P1[:, :], W1[:, :], G[:, NI:2 * NI], ALU.mult)
                        Ps.append((P0, P1))

                    outC = outc_pool.tile([64, NI], F32, tag="outC")
                    for bs in range(0, NI, 512):
                        blen = min(512, NI - bs)
                        ps = psum_pool.tile([64, 512], F32, tag="ops")
                        for ref in range(N):
                            P0, P1 = Ps[ref]
                            nc.tensor.matmul(ps[:, :blen], wreg_t[:, :], P0[:, bs:bs + blen],
                                             start=(ref == 0), stop=False)
                            nc.tensor.matmul(ps[:, :blen], wreg_t[:, :], P1[:, bs:bs + blen],
                                             start=False, stop=(ref == N - 1))
                        nc.vector.tensor_copy(outC[:, bs:bs + blen], ps[:, :blen])
                    _emit_out_dmas(nc, out_d, outC, base, base + NI, 64)

    nc.compile()
    return nc


def _host_prep(x, w_off, b_off, w_mod, b_mod, w_reg):
    xb = x.astype(ml_dtypes.bfloat16)

    xpad = np.zeros((B, 2, 128, H + 2, WP), dtype=ml_dtypes.bfloat16)
    xpad[:, :, :, 1:H + 1, 1:W + 1] = xb.reshape(B, 2, 128, H, W)

    HR = H + 2 * PAD
    xq = np.zeros((B, N, C, HR + 1, TW + 1), dtype=ml_dtypes.bfloat16)
    xq[:, :, :, PAD:PAD + H, PAD:PAD + W] = xb
    q = np.stack([
        xq[:, :, :, :-1, :-1], xq[:, :, :, :-1, 1:],
        xq[:, :, :, 1:, :-1], xq[:, :, :, 1:, 1:],
    ], axis=3)  # [B, N, C, 4, HR, TW]
    q = np.ascontiguousarray(q.transpose(0, 1, 4, 5, 3, 2))  # [B, N, HR, TW, 4, C]

    wcat = np.concatenate([w_off, w_mod], axis=0)  # [12, 256, 3, 3]
    wconv = (wcat.reshape(12, 2, 128, 3, 3).transpose(2, 1, 3, 4, 0)
             .reshape(128, 2 * 9 * 12))
    wconv = np.ascontiguousarray(wconv).astype(ml_dtypes.bfloat16)

    wreg = np.concatenate([w_reg[:, :, 0, 0].T, w_reg[:, :, 0, 0].T], axis=0) * 0.5
    wreg = np.ascontiguousarray(wreg).astype(ml_dtypes.bfloat16)

    flat = np.arange(PXP, dtype=np.int64)
    yloc = (flat // WP).astype(np.float32)
    xloc = (flat % WP).astype(np.float32)
    gy = np.ascontiguousarray(yloc.reshape(147, 128).T)
    gx = np.ascontiguousarray(xloc.reshape(147, 128).T)

    in_maps = []
    for b in range(B):
        for half in range(2):
            r0 = half * RH
            xc = np.ascontiguousarray(
                xpad[b, :, :, r0:r0 + XCROWS, :].reshape(2, 128, XCF))
            qs = np.ascontiguousarray(
                q[b, :, r0:r0 + TROWS, :, :, :].reshape(N, TN, 256))
            in_maps.append({
                "xc": xc, "q": qs, "wconv": wconv, "wreg": wreg,
                "gy": gy, "gx": gx,
            })
    return in_maps


_CACHE = {}


def kernel(x, w_off, b_off, w_mod, b_mod, w_reg):
    x = np.asarray(x, np.float32)
    w_off = np.asarray(w_off, np.float32)
    b_off = np.asarray(b_off, np.float32)
    w_mod = np.asarray(w_mod, np.float32)
    b_mod = np.asarray(b_mod, np.float32)
    w_reg = np.asarray(w_reg, np.float32)

    key = (b_off.tobytes(), b_mod.tobytes())
    if key not in _CACHE:
        _CACHE[key] = _build_program(b_off, b_mod)
    nc = _CACHE[key]

    in_maps = _host_prep(x, w_off, b_off, w_mod, b_mod, w_reg)
    res = run_bass_kernel_spmd(nc, in_maps, core_ids=list(range(8)))
    global LAST_EXEC_NS
    if getattr(res, "exec_time_ns", None):
        LAST_EXEC_NS = res.exec_time_ns

    out = np.zeros((B, 64, H, W), np.float32)
    offset = np.zeros((B, 8, H, W), np.float32)
    for ci, r in enumerate(res.results):
        b, half = ci // 2, ci % 2
        r0 = half * RH
        out[b, :, r0:r0 + RH, :] = np.asarray(r["out"]).reshape(64, RH, W)
        offset[b, :, r0:r0 + RH, :] = np.asarray(r["off"]).reshape(8, RH, W)
    return out, offset
